# revision 1
# baseline (speedup 1.0000x reference)
"""Trainium2 Bass kernel for the attention module:

    att_h  = h @ W_h2att.T + b_h2att             # [B, 512]
    dot    = tanh(p_att_feats + att_h[:, None])  # [B, 1024, 512]
    scores = dot @ w_alpha + b_alpha             # [B, 1024]
    weight = softmax(scores, axis=1)
    out    = einsum('bs,bsd->bd', weight, att_feats)  # [B, 2048]

Sharding: data-parallel over batch B=64 across 8 NeuronCores (8 per core).
Params are tiny and replicated. b_alpha is a softmax shift -> dropped.

Per-core structure (b-major pipeline, all fp32):
  setup: att_h via TensorE (host-transposed W.T/h.T), broadcast rows via
         DRAM bounce + partition_broadcast
  per b: score tiles (DVE add + ScalarE tanh + DVE mul/reduce)
         -> per-b softmax in [t, s-in-tile] layout (TensorE transposes for
            partition reductions, exp accum_out for the denominator)
         -> unnormalized weighted sum via M=1 PSUM-accumulating matmuls
         -> normalize the [1, 2048] row by 1/Z, DMA out
"""

import numpy as np

import concourse.bass as bass
import concourse.tile as tile
from concourse import bacc, mybir
from concourse.bass import ts
from concourse.bass_utils import run_bass_kernel_spmd

F32 = mybir.dt.float32
F32R = mybir.dt.float32r

B_LOC = 8       # batches per core
S = 1024        # attended positions
ST = S // 128   # 8 s-tiles of 128
G = 2           # s-tiles per DMA group
NG = ST // G    # 4 groups
HID = 512
D = 2048
DT = D // 512   # 4 output column slices
K = 2048        # rnn_size (contraction for att_h)
KT = K // 128   # 16 k-tiles

_NC_CACHE = None


def build_kernel(att_bufs=8):
    nc = bacc.Bacc("TRN2", target_bir_lowering=False, debug=False, num_devices=8)

    p_d = nc.dram_tensor("p", [B_LOC, S, HID], F32, kind="ExternalInput")
    att_d = nc.dram_tensor("att", [B_LOC, S, D], F32, kind="ExternalInput")
    hT_d = nc.dram_tensor("hT", [K, B_LOC], F32, kind="ExternalInput")
    WT_d = nc.dram_tensor("WT", [K, HID], F32, kind="ExternalInput")
    wab_d = nc.dram_tensor("wab", [128, HID], F32, kind="ExternalInput")
    bias8_d = nc.dram_tensor("bias8", [B_LOC, HID], F32, kind="ExternalInput")
    ident_d = nc.dram_tensor("ident", [128, 128], F32, kind="ExternalInput")
    out_d = nc.dram_tensor("out", [B_LOC, D], F32, kind="ExternalOutput")
    scratch_d = nc.dram_tensor("atth_scratch", [B_LOC, HID], F32)

    with tile.TileContext(nc) as tc:
        with (
            tc.tile_pool(name="consts", bufs=1) as consts,
            tc.tile_pool(name="singles", bufs=1) as singles,
            tc.tile_pool(name="wt", bufs=2) as wtpool,
            tc.tile_pool(name="ht", bufs=3) as htpool,
            tc.tile_pool(name="ahbc", bufs=B_LOC) as ahbcpool,
            tc.tile_pool(name="pp", bufs=3) as ppool,
            tc.tile_pool(name="th", bufs=3) as thpool,
            tc.tile_pool(name="sct", bufs=2) as sctpool,
            tc.tile_pool(name="small", bufs=2) as smallpool,
            tc.tile_pool(name="wgtp", bufs=2) as wgtpool,
            tc.tile_pool(name="attp", bufs=att_bufs) as attpool,
            tc.tile_pool(name="rowp", bufs=2) as rowpool,
            tc.tile_pool(name="ps_setup", bufs=1, space=bass.MemorySpace.PSUM) as ps_setup,
            tc.tile_pool(name="ps_tp", bufs=3, space=bass.MemorySpace.PSUM) as ps_tp,
            tc.tile_pool(name="ps_acc", bufs=4, space=bass.MemorySpace.PSUM) as ps_acc,
        ):
            # ---- constants ----
            wab = consts.tile([128, HID], F32)
            nc.sync.dma_start(wab[:], wab_d[:])
            ident = consts.tile([128, 128], F32)
            nc.sync.dma_start(ident[:], ident_d[:])
            bias8 = consts.tile([B_LOC, HID], F32)
            nc.sync.dma_start(bias8[:], bias8_d[:])

            # ---- att_h = h @ W.T + b  ([8, 512]) ----
            atth_ps = ps_setup.tile([B_LOC, HID], F32)
            KJ = 2
            KG = KT // KJ
            WT_r = WT_d.rearrange("(kg q j) h -> kg q j h", q=128, j=KJ)
            hT_r = hT_d.rearrange("(kg q j) h -> kg q j h", q=128, j=KJ)
            for kg in range(KG):
                wt = wtpool.tile([128, KJ, HID], F32)
                nc.sync.dma_start(wt[:], WT_r[kg])
                ht = htpool.tile([128, KJ, B_LOC], F32)
                nc.sync.dma_start(ht[:], hT_r[kg])
                for j in range(KJ):
                    nc.tensor.matmul(
                        atth_ps[:], ht[:, j, :], wt[:, j, :],
                        start=(kg == 0 and j == 0),
                        stop=(kg == KG - 1 and j == KJ - 1),
                    )
            A = singles.tile([B_LOC, HID], F32)
            nc.scalar.copy(A[:], atth_ps[:])
            A2 = singles.tile([B_LOC, HID], F32)
            nc.vector.tensor_add(A2[:], A[:], bias8[:])

            # broadcast att_h rows across 128 partitions
            nc.sync.dma_start(scratch_d[:], A2[:])
            ahbc = []
            for b in range(B_LOC):
                row = rowpool.tile([1, HID], F32, name=f"ahrow{b}", tag="ahrow")
                nc.sync.dma_start(row[:], scratch_d[b : b + 1, :])
                t = ahbcpool.tile([128, HID], F32, name=f"ahbc{b}", tag="ahbc")
                nc.gpsimd.partition_broadcast(t[:], row[:])
                ahbc.append(t)

            p_r = [
                p_d[b].rearrange("(g q j) h -> g q j h", q=128, j=G)
                for b in range(B_LOC)
            ]
            att_r = [
                att_d[b].rearrange("(g q j) h -> g q j h", q=128, j=G)
                for b in range(B_LOC)
            ]

            wgtT = {}
            rzs = {}

            def emit_scores(b):
                # ---- scores for batch b: sc_b[s_in_tile, col] ----
                sc_b = sctpool.tile([128, ST], F32, name=f"sc{b}", tag="sc")
                for g in range(NG):
                    pt = ppool.tile([128, G, HID], F32, name=f"pt{b}_{g}", tag="pt")
                    nc.sync.dma_start(pt[:], p_r[b][g])
                    nc.vector.tensor_add(
                        pt[:], pt[:],
                        ahbc[b][:, None, :].broadcast_to((128, G, HID)),
                    )
                    th = thpool.tile([128, G, HID], F32, name=f"th{b}_{g}", tag="th")
                    nc.scalar.activation(
                        th[:], pt[:], mybir.ActivationFunctionType.Tanh
                    )
                    nc.vector.tensor_mul(
                        th[:], th[:],
                        wab[:, None, :].broadcast_to((128, G, HID)),
                    )
                    nc.vector.reduce_sum(
                        sc_b[:, ts(g, G)], th[:], axis=mybir.AxisListType.X
                    )

                # ---- per-b softmax (weights left unnormalized) ----
                tp1 = ps_tp.tile([ST, 128], F32, name=f"tp1_{b}", tag="tp")
                nc.tensor.transpose(tp1[:], sc_b[:], ident[:])
                Sb = smallpool.tile([ST, 128], F32, name=f"Sb{b}", tag="Sb")
                nc.scalar.copy(Sb[:], tp1[:])
                m8 = smallpool.tile([ST, 1], F32, name=f"m8{b}", tag="m8")
                nc.vector.reduce_max(m8[:], Sb[:], axis=mybir.AxisListType.X)
                tp2 = ps_tp.tile([1, ST], F32, name=f"tp2_{b}", tag="tp")
                nc.tensor.transpose(tp2[:], m8[:], ident[:ST, :ST])
                m1 = smallpool.tile([1, ST], F32, name=f"m1{b}", tag="m1")
                nc.scalar.copy(m1[:], tp2[:])
                gmneg = smallpool.tile([1, 1], F32, name=f"gm{b}", tag="gm")
                nc.vector.reduce_max(
                    gmneg[:], m1[:], axis=mybir.AxisListType.X, negate=True
                )
                gm8 = smallpool.tile([ST, 1], F32, name=f"gm8{b}", tag="gm8")
                nc.gpsimd.partition_broadcast(gm8[:], gmneg[:])
                Eb = smallpool.tile([ST, 128], F32, name=f"Eb{b}", tag="Eb")
                z8 = smallpool.tile([ST, 1], F32, name=f"z8{b}", tag="z8")
                nc.scalar.activation(
                    Eb[:], Sb[:], mybir.ActivationFunctionType.Exp,
                    bias=gm8[:], accum_out=z8[:],
                )
                tp3 = ps_tp.tile([1, ST], F32, name=f"tp3_{b}", tag="tp")
                nc.tensor.transpose(tp3[:], z8[:], ident[:ST, :ST])
                z1 = smallpool.tile([1, ST], F32, name=f"z1{b}", tag="z1")
                nc.scalar.copy(z1[:], tp3[:])
                Z = smallpool.tile([1, 1], F32, name=f"Z{b}", tag="Z")
                nc.vector.reduce_sum(Z[:], z1[:], axis=mybir.AxisListType.X)
                rz = smallpool.tile([1, 1], F32, name=f"rz{b}", tag="rz")
                nc.vector.reciprocal(rz[:], Z[:])
                rzs[b] = rz
                tp4 = ps_tp.tile([128, ST], F32, name=f"tp4_{b}", tag="tp")
                nc.tensor.transpose(tp4[:], Eb[:], ident[:ST, :ST])
                w_sb = wgtpool.tile([128, ST], F32R, name=f"wgtT{b}", tag="wgtT")
                nc.scalar.copy(w_sb[:], tp4[:])
                wgtT[b] = w_sb

            def emit_weighted(b):
                accs = [
                    ps_acc.tile([1, 512], F32, name=f"acc{b}_{d}", tag="acc")
                    for d in range(DT)
                ]
                for g in range(NG):
                    at = attpool.tile([128, G, D], F32R, name=f"at{b}_{g}", tag="at")
                    nc.sync.dma_start(at[:], att_r[b][g].bitcast(F32R))
                    for u in range(G):
                        t = g * G + u
                        for d in range(DT):
                            nc.tensor.matmul(
                                accs[d][:],
                                wgtT[b][:, t : t + 1],
                                at[:, u, ts(d, 512)],
                                start=(t == 0),
                                stop=(t == ST - 1),
                            )
                rowbuf = rowpool.tile([1, D], F32, name=f"row{b}", tag="rowbuf")
                for d in range(DT):
                    nc.scalar.copy(rowbuf[0:1, ts(d, 512)], accs[d][:])
                nc.vector.tensor_scalar_mul(rowbuf[:], rowbuf[:], rzs[b][:])
                nc.sync.dma_start(out_d[b : b + 1, :], rowbuf[:])

            emit_scores(0)
            for b in range(B_LOC):
                if b + 1 < B_LOC:
                    emit_scores(b + 1)
                emit_weighted(b)

    nc.compile()
    return nc


def _in_maps(h, att_feats, p_att_feats, W_h2att, b_h2att, w_alpha):
    WT = np.ascontiguousarray(W_h2att.T).astype(np.float32)
    wab = np.ascontiguousarray(
        np.broadcast_to(w_alpha.astype(np.float32), (128, HID))
    )
    bias8 = np.ascontiguousarray(
        np.broadcast_to(b_h2att.astype(np.float32), (B_LOC, HID))
    )
    ident = np.eye(128, dtype=np.float32)
    maps = []
    for c in range(8):
        sl = slice(c * B_LOC, (c + 1) * B_LOC)
        maps.append(
            {
                "p": np.ascontiguousarray(p_att_feats[sl]).astype(np.float32),
                "att": np.ascontiguousarray(att_feats[sl]).astype(np.float32),
                "hT": np.ascontiguousarray(h[sl].T).astype(np.float32),
                "WT": WT,
                "wab": wab,
                "bias8": bias8,
                "ident": ident,
            }
        )
    return maps


def kernel(h, att_feats, p_att_feats, W_h2att, b_h2att, w_alpha, b_alpha):
    global _NC_CACHE
    h = np.asarray(h)
    att_feats = np.asarray(att_feats)
    p_att_feats = np.asarray(p_att_feats)
    W_h2att = np.asarray(W_h2att)
    b_h2att = np.asarray(b_h2att)
    w_alpha = np.asarray(w_alpha)
    if _NC_CACHE is None:
        _NC_CACHE = build_kernel()
    nc = _NC_CACHE
    maps = _in_maps(h, att_feats, p_att_feats, W_h2att, b_h2att, w_alpha)
    res = run_bass_kernel_spmd(nc, maps, core_ids=list(range(8)))
    out = np.concatenate([res.results[c]["out"] for c in range(8)], axis=0)
    return out.astype(np.float32)



# revision 7
# speedup vs baseline: 1.6006x; 1.6006x over previous
"""Trainium2 Bass kernel for the attention module:

    att_h  = h @ W_h2att.T + b_h2att             # [B, 512]
    dot    = tanh(p_att_feats + att_h[:, None])  # [B, 1024, 512]
    scores = dot @ w_alpha + b_alpha             # [B, 1024]
    weight = softmax(scores, axis=1)
    out    = einsum('bs,bsd->bd', weight, att_feats)  # [B, 2048]

Sharding: data-parallel over batch B=64 across 8 NeuronCores (8 per core).
Params are tiny and replicated. b_alpha is a softmax shift -> dropped.

v2 design (DMA-bound problem; ~42MB/core HBM read):
  - All large tensors (p, att, W) are cast to bf16 on the host; rel err
    budget is 2e-2 and the all-bf16 pipeline measures ~2.6e-3.
  - Two HWDGE rings: p-stream + setup on ACT (nc.scalar), att-stream on
    SP (nc.sync) so neither stream head-of-line-blocks the other.
  - scores: DVE add (bf16) -> ACT tanh (bf16) -> DVE in-place mul by
    w_alpha -> DVE reduce_sum into score columns (all-bf16 operands for
    2x DVE throughput; tensor_tensor_reduce crashes this runtime).
  - softmax without max-subtraction (|scores| <= ||w_alpha||_1 ~ 18, exp
    can't overflow fp32/bf16): one ACT Exp with accum_out gives both the
    bf16 weight tile (already in matmul lhsT layout) and per-partition
    partial sums; one PE transpose + DVE reduce gives 1/Z.
  - att_h row-broadcast via ones-vector TensorE matmul (no DRAM bounce).
  - weighted sum: per-b M=1 PSUM-accumulating bf16 matmuls, one fused
    output DMA at the end.

s-index mapping (shared by p, scores, weights, att):
    s = g*512 + q*4 + j   (g: group 0..1, q: partition 0..127, j: 0..3)
"""

import numpy as np
import ml_dtypes

import concourse.bass as bass
import concourse.tile as tile
from concourse import bacc, mybir
from concourse.bass import ts
from concourse.bass_utils import run_bass_kernel_spmd

F32 = mybir.dt.float32
BF16 = mybir.dt.bfloat16

B_LOC = 8       # batches per core
S = 1024        # attended positions
J = 4           # s per (group, partition)
NG = 2          # s groups
ST = NG * J     # 8 score columns
HID = 512
D = 2048
DT = D // 512   # 4 output column slices
K = 2048        # rnn_size (contraction for att_h)
KJ = 2
KG = K // (128 * KJ)  # 8 k-groups

_NC_CACHE = None


def build_kernel(att_bufs=7):
    nc = bacc.Bacc("TRN2", target_bir_lowering=False, debug=False, num_devices=8)

    p_d = nc.dram_tensor("p", [B_LOC, S, HID], BF16, kind="ExternalInput")
    att_d = nc.dram_tensor("att", [B_LOC, S, D], BF16, kind="ExternalInput")
    hT_d = nc.dram_tensor("hT", [K, B_LOC], BF16, kind="ExternalInput")
    WT_d = nc.dram_tensor("WT", [K, HID], BF16, kind="ExternalInput")
    wab_d = nc.dram_tensor("wab", [128, HID], BF16, kind="ExternalInput")
    bias8_d = nc.dram_tensor("bias8", [B_LOC, HID], F32, kind="ExternalInput")
    ident_d = nc.dram_tensor("ident", [128, 128], F32, kind="ExternalInput")
    sel_d = nc.dram_tensor("sel", [B_LOC, B_LOC * 128], BF16, kind="ExternalInput")
    out_d = nc.dram_tensor("out", [B_LOC, D], F32, kind="ExternalOutput")

    with tile.TileContext(nc) as tc:
        with (
            tc.tile_pool(name="consts", bufs=1) as consts,
            tc.tile_pool(name="singles", bufs=1) as singles,
            tc.tile_pool(name="ahbc", bufs=B_LOC) as ahbcpool,
            tc.tile_pool(name="pp", bufs=3) as ppool,
            tc.tile_pool(name="pa", bufs=3) as papool,
            tc.tile_pool(name="th", bufs=3) as thpool,
            tc.tile_pool(name="sct", bufs=3) as sctpool,
            tc.tile_pool(name="wgtp", bufs=3) as wgtpool,
            tc.tile_pool(name="zp", bufs=2) as zpool,
            tc.tile_pool(name="small", bufs=2) as smallpool,
            tc.tile_pool(name="row", bufs=2) as rowpool,
            tc.tile_pool(name="attp", bufs=att_bufs) as attpool,
            tc.tile_pool(name="ps_setup", bufs=1, space=bass.MemorySpace.PSUM) as ps_setup,
            tc.tile_pool(name="ps_bc", bufs=2, space=bass.MemorySpace.PSUM) as ps_bc,
            tc.tile_pool(name="ps_z", bufs=1, space=bass.MemorySpace.PSUM) as ps_z,
            tc.tile_pool(name="ps_acc", bufs=4, space=bass.MemorySpace.PSUM) as ps_acc,
        ):
            # ---- constants (ACT ring) ----
            wab = consts.tile([128, HID], BF16)
            nc.scalar.dma_start(wab[:], wab_d[:])
            ident = consts.tile([128, 128], F32)
            nc.scalar.dma_start(ident[:], ident_d[:])
            sel = consts.tile([B_LOC, B_LOC * 128], BF16)
            nc.scalar.dma_start(sel[:], sel_d[:])
            bias8 = consts.tile([B_LOC, HID], F32)
            nc.scalar.dma_start(bias8[:], bias8_d[:])

            # ---- att_h = h @ W.T + b  ([8, 512]) ----
            wt_all = consts.tile([128, KG, KJ, HID], BF16)
            nc.scalar.dma_start(
                wt_all[:], WT_d.rearrange("(kg q j) h -> q kg j h", q=128, j=KJ)
            )
            ht_all = consts.tile([128, KG, KJ, B_LOC], BF16)
            nc.scalar.dma_start(
                ht_all[:], hT_d.rearrange("(kg q j) h -> q kg j h", q=128, j=KJ)
            )
            atth_ps = ps_setup.tile([B_LOC, HID], F32)
            for kg in range(KG):
                for j in range(KJ):
                    nc.tensor.matmul(
                        atth_ps[:], ht_all[:, kg, j, :], wt_all[:, kg, j, :],
                        start=(kg == 0 and j == 0),
                        stop=(kg == KG - 1 and j == KJ - 1),
                    )
            A2 = singles.tile([B_LOC, HID], BF16)
            nc.vector.tensor_add(A2[:], atth_ps[:], bias8[:])

            ahbc = [None] * B_LOC

            def emit_bcast(b):
                # broadcast att_h row b across 128 partitions via ones-matmul
                bc = ps_bc.tile([128, HID], F32, name=f"bc{b}", tag="bc")
                nc.tensor.matmul(
                    bc[:], sel[:, b * 128 : (b + 1) * 128], A2[:],
                    start=True, stop=True,
                )
                t = ahbcpool.tile([128, HID], BF16, name=f"ahbc{b}", tag="ahbc")
                nc.scalar.copy(t[:], bc[:])
                ahbc[b] = t

            p_r = [
                p_d[b].rearrange("(g q j) h -> g q j h", q=128, j=J)
                for b in range(B_LOC)
            ]
            att_r = [
                att_d[b].rearrange("(g q j) h -> g q j h", q=128, j=J)
                for b in range(B_LOC)
            ]

            wgtT = {}
            rzs = {}

            def emit_scores(b):
                sc_b = sctpool.tile([128, ST], F32, name=f"sc{b}", tag="sc")
                for g in range(NG):
                    pt = ppool.tile([128, J, HID], BF16, name=f"pt{b}_{g}", tag="pt")
                    nc.scalar.dma_start(pt[:], p_r[b][g])
                    pa = papool.tile([128, J, HID], BF16, name=f"pa{b}_{g}", tag="pa")
                    nc.vector.tensor_add(
                        pa[:], pt[:],
                        ahbc[b][:, None, :].broadcast_to((128, J, HID)),
                    )
                    th = thpool.tile([128, J, HID], BF16, name=f"th{b}_{g}", tag="th")
                    nc.scalar.activation(
                        th[:], pa[:], mybir.ActivationFunctionType.Tanh
                    )
                    nc.vector.tensor_mul(
                        th[:], th[:],
                        wab[:, None, :].broadcast_to((128, J, HID)),
                    )
                    nc.vector.reduce_sum(
                        sc_b[:, ts(g, J)], th[:], axis=mybir.AxisListType.X
                    )

                # softmax without max-subtraction; weights left unnormalized
                wgt = wgtpool.tile([128, ST], BF16, name=f"wgt{b}", tag="wgt")
                z128 = zpool.tile([128, 1], F32, name=f"z{b}", tag="z")
                nc.scalar.activation(
                    wgt[:], sc_b[:], mybir.ActivationFunctionType.Exp,
                    accum_out=z128[:],
                )
                tpz = ps_z.tile([1, 128], F32, name=f"tpz{b}", tag="tpz")
                nc.tensor.transpose(tpz[:], z128[:], ident[:])
                Z = smallpool.tile([1, 1], F32, name=f"Z{b}", tag="Z")
                nc.vector.reduce_sum(Z[:], tpz[:], axis=mybir.AxisListType.X)
                rz = smallpool.tile([1, 1], F32, name=f"rz{b}", tag="rz")
                nc.vector.reciprocal(rz[:], Z[:])
                rzs[b] = rz
                wgtT[b] = wgt

            def emit_weighted(b):
                accs = [
                    ps_acc.tile([1, 512], F32, name=f"acc{b}_{d}", tag="acc")
                    for d in range(DT)
                ]
                for g in range(NG):
                    at = attpool.tile([128, J, D], BF16, name=f"at{b}_{g}", tag="at")
                    nc.sync.dma_start(at[:], att_r[b][g])
                    for u in range(J):
                        t = g * J + u
                        for d in range(DT):
                            nc.tensor.matmul(
                                accs[d][:],
                                wgtT[b][:, t : t + 1],
                                at[:, u, ts(d, 512)],
                                start=(t == 0),
                                stop=(t == ST - 1),
                            )
                rowbuf = rowpool.tile([1, D], F32, name=f"row{b}", tag="row")
                for d in range(DT):
                    nc.scalar.copy(rowbuf[0:1, ts(d, 512)], accs[d][:])
                nc.vector.tensor_scalar_mul(rowbuf[:], rowbuf[:], rzs[b][:])
                nc.scalar.dma_start(out_d[b : b + 1, :], rowbuf[:])

            # staggered emission: broadcasts and scores stay ~2 batches
            # ahead of the weighted-sum stream
            emit_bcast(0)
            emit_scores(0)
            emit_bcast(1)
            emit_scores(1)
            for b in range(B_LOC):
                if b + 2 < B_LOC:
                    emit_bcast(b + 2)
                    emit_scores(b + 2)
                emit_weighted(b)

    nc.compile()
    return nc


def _in_maps(h, att_feats, p_att_feats, W_h2att, b_h2att, w_alpha):
    bf = ml_dtypes.bfloat16
    att_bf = np.ascontiguousarray(att_feats).astype(bf)
    p_bf = np.ascontiguousarray(p_att_feats).astype(bf)
    WT = np.ascontiguousarray(W_h2att.T).astype(bf)
    wab = np.ascontiguousarray(
        np.broadcast_to(w_alpha.astype(np.float32), (128, HID))
    ).astype(bf)
    bias8 = np.ascontiguousarray(
        np.broadcast_to(b_h2att.astype(np.float32), (B_LOC, HID))
    )
    ident = np.eye(128, dtype=np.float32)
    sel = np.kron(np.eye(B_LOC, dtype=np.float32), np.ones((1, 128), dtype=np.float32)).astype(ml_dtypes.bfloat16)
    maps = []
    for c in range(8):
        sl = slice(c * B_LOC, (c + 1) * B_LOC)
        maps.append(
            {
                "p": np.ascontiguousarray(p_bf[sl]),
                "att": np.ascontiguousarray(att_bf[sl]),
                "hT": np.ascontiguousarray(h[sl].T.astype(bf)),
                "WT": WT,
                "wab": wab,
                "bias8": bias8,
                "ident": ident,
                "sel": sel,
            }
        )
    return maps


def kernel(h, att_feats, p_att_feats, W_h2att, b_h2att, w_alpha, b_alpha):
    global _NC_CACHE
    h = np.asarray(h)
    att_feats = np.asarray(att_feats)
    p_att_feats = np.asarray(p_att_feats)
    W_h2att = np.asarray(W_h2att)
    b_h2att = np.asarray(b_h2att)
    w_alpha = np.asarray(w_alpha)
    if _NC_CACHE is None:
        _NC_CACHE = build_kernel()
    nc = _NC_CACHE
    maps = _in_maps(h, att_feats, p_att_feats, W_h2att, b_h2att, w_alpha)
    res = run_bass_kernel_spmd(nc, maps, core_ids=list(range(8)))
    out = np.concatenate([res.results[c]["out"] for c in range(8)], axis=0)
    return out.astype(np.float32)


# revision 8
# speedup vs baseline: 1.9579x; 1.2232x over previous
"""Trainium2 Bass kernel for the attention module:

    att_h  = h @ W_h2att.T + b_h2att             # [B, 512]
    dot    = tanh(p_att_feats + att_h[:, None])  # [B, 1024, 512]
    scores = dot @ w_alpha + b_alpha             # [B, 1024]
    weight = softmax(scores, axis=1)
    out    = einsum('bs,bsd->bd', weight, att_feats)  # [B, 2048]

Sharding: data-parallel over batch B=64 across 8 NeuronCores (8 per core).
Params are tiny and replicated. b_alpha is a softmax shift -> dropped.

v2c design (DMA-bound problem; ~44MB/core HBM read):
  - All large tensors (p, att, W) are cast to bf16 on the host; rel err
    budget is 2e-2 and the all-bf16 pipeline measures ~3e-3.
  - All input DMAs on the SP HWDGE ring (sync engine is otherwise idle,
    so DMA issue never blocks compute); tiny output DMAs on the ACT ring.
  - scores: DVE in-place add (bf16) -> ACT tanh (bf16) -> DVE in-place
    mul by w_alpha -> DVE reduce_sum into score columns.
  - softmax without max-subtraction (|scores| <= ||w_alpha||_1 ~ 18, exp
    can't overflow fp32): one ACT Exp per batch emits the bf16 weight
    tile (already in matmul lhsT layout) and f32 per-partition partial
    sums into a column of zall; Z-reduction and 1/Z normalization happen
    on the host (64 divides).
  - att_h row-broadcast via selector-matrix TensorE matmul (no DRAM
    bounce, no gpsimd).
  - weighted sum: per-b M=1 PSUM-accumulating bf16 matmuls; unnormalized
    rows + zall DMA'd out.

s-index mapping (shared by p, scores, weights, att):
    s = g*512 + q*4 + j   (g: group 0..1, q: partition 0..127, j: 0..3)
"""

import numpy as np
import ml_dtypes

import concourse.bass as bass
import concourse.tile as tile
from concourse import bacc, mybir
from concourse.bass import ts
from concourse.bass_utils import run_bass_kernel_spmd

F32 = mybir.dt.float32
BF16 = mybir.dt.bfloat16

B_LOC = 8       # batches per core
S = 1024        # attended positions
J = 4           # s per (group, partition)
NG = 2          # s groups
ST = NG * J     # 8 score columns
HID = 512
D = 2048
DT = D // 512   # 4 output column slices
K = 2048        # rnn_size (contraction for att_h)
KJ = 2
KG = K // (128 * KJ)  # 8 k-groups

_NC_CACHE = None


def build_kernel(att_bufs=6, p_bufs=6):
    nc = bacc.Bacc("TRN2", target_bir_lowering=False, debug=False, num_devices=8)

    p_d = nc.dram_tensor("p", [B_LOC, S, HID], BF16, kind="ExternalInput")
    att_d = nc.dram_tensor("att", [B_LOC, S, D], BF16, kind="ExternalInput")
    hT_d = nc.dram_tensor("hT", [K, B_LOC], BF16, kind="ExternalInput")
    WT_d = nc.dram_tensor("WT", [K, HID], BF16, kind="ExternalInput")
    wab_d = nc.dram_tensor("wab", [128, HID], BF16, kind="ExternalInput")
    bias8_d = nc.dram_tensor("bias8", [B_LOC, HID], F32, kind="ExternalInput")
    sel_d = nc.dram_tensor("sel", [B_LOC, B_LOC * 128], BF16, kind="ExternalInput")
    out_d = nc.dram_tensor("out", [B_LOC, D], F32, kind="ExternalOutput")
    z_d = nc.dram_tensor("zall", [128, B_LOC], F32, kind="ExternalOutput")

    with tile.TileContext(nc) as tc:
        with (
            tc.tile_pool(name="consts", bufs=1) as consts,
            tc.tile_pool(name="singles", bufs=1) as singles,
            tc.tile_pool(name="ahbc", bufs=B_LOC) as ahbcpool,
            tc.tile_pool(name="pp", bufs=p_bufs) as ppool,
            tc.tile_pool(name="th", bufs=3) as thpool,
            tc.tile_pool(name="sct", bufs=3) as sctpool,
            tc.tile_pool(name="wgtp", bufs=3) as wgtpool,
            tc.tile_pool(name="row", bufs=2) as rowpool,
            tc.tile_pool(name="attp", bufs=att_bufs) as attpool,
            tc.tile_pool(name="ps_setup", bufs=1, space=bass.MemorySpace.PSUM) as ps_setup,
            tc.tile_pool(name="ps_bc", bufs=2, space=bass.MemorySpace.PSUM) as ps_bc,
            tc.tile_pool(name="ps_acc", bufs=5, space=bass.MemorySpace.PSUM) as ps_acc,
        ):
            # ---- setup DMAs (SP ring, ahead of the p/att stream) ----
            wt_all = consts.tile([128, KG, KJ, HID], BF16)
            nc.sync.dma_start(
                wt_all[:], WT_d.rearrange("(kg q j) h -> q kg j h", q=128, j=KJ)
            )
            ht_all = consts.tile([128, KG, KJ, B_LOC], BF16)
            nc.sync.dma_start(
                ht_all[:], hT_d.rearrange("(kg q j) h -> q kg j h", q=128, j=KJ)
            )
            sel = consts.tile([B_LOC, B_LOC * 128], BF16)
            nc.sync.dma_start(sel[:], sel_d[:])
            wab = consts.tile([128, HID], BF16)
            nc.sync.dma_start(wab[:], wab_d[:])
            bias8 = consts.tile([B_LOC, HID], F32)
            nc.sync.dma_start(bias8[:], bias8_d[:])

            # ---- att_h = h @ W.T + b  ([8, 512]) ----
            atth_ps = ps_setup.tile([B_LOC, HID], F32)
            for kg in range(KG):
                for j in range(KJ):
                    nc.tensor.matmul(
                        atth_ps[:], ht_all[:, kg, j, :], wt_all[:, kg, j, :],
                        start=(kg == 0 and j == 0),
                        stop=(kg == KG - 1 and j == KJ - 1),
                    )
            A2 = singles.tile([B_LOC, HID], BF16)
            nc.vector.tensor_add(A2[:], atth_ps[:], bias8[:])

            # per-partition exp partial sums, one column per batch
            zall = singles.tile([128, B_LOC], F32)

            ahbc = [None] * B_LOC

            def emit_bcast(b):
                # broadcast att_h row b across 128 partitions: sel_b.T @ A2
                bc = ps_bc.tile([128, HID], F32, name=f"bc{b}", tag="bc")
                nc.tensor.matmul(
                    bc[:], sel[:, b * 128 : (b + 1) * 128], A2[:],
                    start=True, stop=True,
                )
                t = ahbcpool.tile([128, HID], BF16, name=f"ahbc{b}", tag="ahbc")
                nc.scalar.copy(t[:], bc[:])
                ahbc[b] = t

            p_r = [
                p_d[b].rearrange("(g q j) h -> g q j h", q=128, j=J)
                for b in range(B_LOC)
            ]
            att_r = [
                att_d[b].rearrange("(g q j) h -> g q j h", q=128, j=J)
                for b in range(B_LOC)
            ]

            wgtT = {}

            def emit_scores(b):
                sc_b = sctpool.tile([128, ST], F32, name=f"sc{b}", tag="sc")
                for g in range(NG):
                    pt = ppool.tile([128, J, HID], BF16, name=f"pt{b}_{g}", tag="pt")
                    nc.sync.dma_start(pt[:], p_r[b][g])
                    nc.vector.tensor_add(
                        pt[:], pt[:],
                        ahbc[b][:, None, :].broadcast_to((128, J, HID)),
                    )
                    th = thpool.tile([128, J, HID], BF16, name=f"th{b}_{g}", tag="th")
                    nc.scalar.activation(
                        th[:], pt[:], mybir.ActivationFunctionType.Tanh
                    )
                    nc.vector.tensor_mul(
                        th[:], th[:],
                        wab[:, None, :].broadcast_to((128, J, HID)),
                    )
                    nc.vector.reduce_sum(
                        sc_b[:, ts(g, J)], th[:], axis=mybir.AxisListType.X
                    )
                wgt = wgtpool.tile([128, ST], BF16, name=f"wgt{b}", tag="wgt")
                nc.scalar.activation(
                    wgt[:], sc_b[:], mybir.ActivationFunctionType.Exp,
                    accum_out=zall[:, b : b + 1],
                )
                wgtT[b] = wgt

            def emit_weighted(b):
                accs = [
                    ps_acc.tile([1, 512], F32, name=f"acc{b}_{d}", tag="acc")
                    for d in range(DT)
                ]
                for g in range(NG):
                    at = attpool.tile([128, J, D], BF16, name=f"at{b}_{g}", tag="at")
                    nc.sync.dma_start(at[:], att_r[b][g])
                    for u in range(J):
                        t = g * J + u
                        for d in range(DT):
                            nc.tensor.matmul(
                                accs[d][:],
                                wgtT[b][:, t : t + 1],
                                at[:, u, ts(d, 512)],
                                start=(t == 0),
                                stop=(t == ST - 1),
                            )
                rowbuf = rowpool.tile([1, D], F32, name=f"row{b}", tag="row")
                for d in range(DT):
                    nc.scalar.copy(rowbuf[0:1, ts(d, 512)], accs[d][:])
                nc.scalar.dma_start(out_d[b : b + 1, :], rowbuf[:])

            # staggered emission: broadcasts and scores stay ~2 batches
            # ahead of the weighted-sum stream
            emit_bcast(0)
            emit_scores(0)
            emit_bcast(1)
            emit_scores(1)
            for b in range(B_LOC):
                if b + 2 < B_LOC:
                    emit_bcast(b + 2)
                    emit_scores(b + 2)
                emit_weighted(b)

            nc.scalar.dma_start(z_d[:], zall[:])

    nc.compile()
    return nc


def _in_maps(h, att_feats, p_att_feats, W_h2att, b_h2att, w_alpha):
    bf = ml_dtypes.bfloat16
    att_bf = np.ascontiguousarray(att_feats).astype(bf)
    p_bf = np.ascontiguousarray(p_att_feats).astype(bf)
    WT = np.ascontiguousarray(W_h2att.T).astype(bf)
    wab = np.ascontiguousarray(
        np.broadcast_to(w_alpha.astype(np.float32), (128, HID))
    ).astype(bf)
    bias8 = np.ascontiguousarray(
        np.broadcast_to(b_h2att.astype(np.float32), (B_LOC, HID))
    )
    sel = np.kron(
        np.eye(B_LOC, dtype=np.float32), np.ones((1, 128), dtype=np.float32)
    ).astype(bf)
    maps = []
    for c in range(8):
        sl = slice(c * B_LOC, (c + 1) * B_LOC)
        maps.append(
            {
                "p": np.ascontiguousarray(p_bf[sl]),
                "att": np.ascontiguousarray(att_bf[sl]),
                "hT": np.ascontiguousarray(h[sl].T.astype(bf)),
                "WT": WT,
                "wab": wab,
                "bias8": bias8,
                "sel": sel,
            }
        )
    return maps


def kernel(h, att_feats, p_att_feats, W_h2att, b_h2att, w_alpha, b_alpha):
    global _NC_CACHE
    h = np.asarray(h)
    att_feats = np.asarray(att_feats)
    p_att_feats = np.asarray(p_att_feats)
    W_h2att = np.asarray(W_h2att)
    b_h2att = np.asarray(b_h2att)
    w_alpha = np.asarray(w_alpha)
    if _NC_CACHE is None:
        _NC_CACHE = build_kernel()
    nc = _NC_CACHE
    maps = _in_maps(h, att_feats, p_att_feats, W_h2att, b_h2att, w_alpha)
    res = run_bass_kernel_spmd(nc, maps, core_ids=list(range(8)))
    outs = []
    for c in range(8):
        row = res.results[c]["out"]                     # [8, 2048] unnormalized
        z = res.results[c]["zall"].sum(axis=0)          # [8]
        outs.append(row / z[:, None])
    return np.concatenate(outs, axis=0).astype(np.float32)


# revision 9
# speedup vs baseline: 1.9851x; 1.0139x over previous
"""Trainium2 Bass kernel for the attention module:

    att_h  = h @ W_h2att.T + b_h2att             # [B, 512]
    dot    = tanh(p_att_feats + att_h[:, None])  # [B, 1024, 512]
    scores = dot @ w_alpha + b_alpha             # [B, 1024]
    weight = softmax(scores, axis=1)
    out    = einsum('bs,bsd->bd', weight, att_feats)  # [B, 2048]

Sharding: data-parallel over batch B=64 across 8 NeuronCores (8 per core).
Params are tiny and replicated. b_alpha is a softmax shift -> dropped.

v2c design (DMA-bound problem; ~44MB/core HBM read):
  - All large tensors (p, att, W) are cast to bf16 on the host; rel err
    budget is 2e-2 and the all-bf16 pipeline measures ~3e-3.
  - All input DMAs on the SP HWDGE ring (sync engine is otherwise idle,
    so DMA issue never blocks compute); tiny output DMAs on the ACT ring.
  - scores: DVE in-place add (bf16) -> ACT tanh (bf16) -> DVE in-place
    mul by w_alpha -> DVE reduce_sum into score columns.
  - softmax without max-subtraction (|scores| <= ||w_alpha||_1 ~ 18, exp
    can't overflow fp32): one ACT Exp per batch emits the bf16 weight
    tile (already in matmul lhsT layout) and f32 per-partition partial
    sums into a column of zall; Z-reduction and 1/Z normalization happen
    on the host (64 divides).
  - att_h row-broadcast via selector-matrix TensorE matmul (no DRAM
    bounce, no gpsimd).
  - weighted sum: per-b M=1 PSUM-accumulating bf16 matmuls; unnormalized
    rows + zall DMA'd out.

s-index mapping (shared by p, scores, weights, att):
    s = g*512 + q*4 + j   (g: group 0..1, q: partition 0..127, j: 0..3)
"""

import numpy as np
import ml_dtypes

import concourse.bass as bass
import concourse.tile as tile
from concourse import bacc, mybir
from concourse.bass import ts
from concourse.bass_utils import run_bass_kernel_spmd

F32 = mybir.dt.float32
BF16 = mybir.dt.bfloat16

B_LOC = 8       # batches per core
S = 1024        # attended positions
J = 4           # s per (group, partition)
NG = 2          # s groups
ST = NG * J     # 8 score columns
HID = 512
D = 2048
DT = D // 512   # 4 output column slices
K = 2048        # rnn_size (contraction for att_h)
KJ = 2
KG = K // (128 * KJ)  # 8 k-groups

_NC_CACHE = None


def build_kernel(att_bufs=12, p_bufs=6):
    nc = bacc.Bacc("TRN2", target_bir_lowering=False, debug=False, num_devices=8)

    p_d = nc.dram_tensor("p", [B_LOC, S, HID], BF16, kind="ExternalInput")
    att_d = nc.dram_tensor("att", [B_LOC, S, D], BF16, kind="ExternalInput")
    hT_d = nc.dram_tensor("hT", [K, B_LOC], BF16, kind="ExternalInput")
    WT_d = nc.dram_tensor("WT", [K, HID], BF16, kind="ExternalInput")
    wab_d = nc.dram_tensor("wab", [128, HID], BF16, kind="ExternalInput")
    bias8_d = nc.dram_tensor("bias8", [B_LOC, HID], F32, kind="ExternalInput")
    sel_d = nc.dram_tensor("sel", [B_LOC, B_LOC * 128], BF16, kind="ExternalInput")
    out_d = nc.dram_tensor("out", [B_LOC, D], F32, kind="ExternalOutput")
    z_d = nc.dram_tensor("zall", [128, B_LOC], F32, kind="ExternalOutput")

    with tile.TileContext(nc) as tc:
        with (
            tc.tile_pool(name="consts", bufs=1) as consts,
            tc.tile_pool(name="singles", bufs=1) as singles,
            tc.tile_pool(name="ahbc", bufs=B_LOC) as ahbcpool,
            tc.tile_pool(name="pp", bufs=p_bufs) as ppool,
            tc.tile_pool(name="th", bufs=3) as thpool,
            tc.tile_pool(name="sct", bufs=3) as sctpool,
            tc.tile_pool(name="wgtp", bufs=3) as wgtpool,
            tc.tile_pool(name="row", bufs=2) as rowpool,
            tc.tile_pool(name="attp", bufs=att_bufs) as attpool,
            tc.tile_pool(name="ps_setup", bufs=1, space=bass.MemorySpace.PSUM) as ps_setup,
            tc.tile_pool(name="ps_bc", bufs=2, space=bass.MemorySpace.PSUM) as ps_bc,
            tc.tile_pool(name="ps_acc", bufs=5, space=bass.MemorySpace.PSUM) as ps_acc,
        ):
            # ---- setup DMAs (SP ring, ahead of the p/att stream) ----
            wt_all = consts.tile([128, KG, KJ, HID], BF16)
            nc.sync.dma_start(
                wt_all[:], WT_d.rearrange("(kg q j) h -> q kg j h", q=128, j=KJ)
            )
            ht_all = consts.tile([128, KG, KJ, B_LOC], BF16)
            nc.sync.dma_start(
                ht_all[:], hT_d.rearrange("(kg q j) h -> q kg j h", q=128, j=KJ)
            )
            sel = consts.tile([B_LOC, B_LOC * 128], BF16)
            nc.sync.dma_start(sel[:], sel_d[:])
            wab = consts.tile([128, HID], BF16)
            nc.sync.dma_start(wab[:], wab_d[:])
            bias8 = consts.tile([B_LOC, HID], F32)
            nc.sync.dma_start(bias8[:], bias8_d[:])

            # ---- att_h = h @ W.T + b  ([8, 512]) ----
            atth_ps = ps_setup.tile([B_LOC, HID], F32)
            for kg in range(KG):
                for j in range(KJ):
                    nc.tensor.matmul(
                        atth_ps[:], ht_all[:, kg, j, :], wt_all[:, kg, j, :],
                        start=(kg == 0 and j == 0),
                        stop=(kg == KG - 1 and j == KJ - 1),
                    )
            A2 = singles.tile([B_LOC, HID], BF16)
            nc.vector.tensor_add(A2[:], atth_ps[:], bias8[:])

            # per-partition exp partial sums, one column per batch
            zall = singles.tile([128, B_LOC], F32)

            ahbc = [None] * B_LOC

            def emit_bcast(b):
                # broadcast att_h row b across 128 partitions: sel_b.T @ A2
                bc = ps_bc.tile([128, HID], F32, name=f"bc{b}", tag="bc")
                nc.tensor.matmul(
                    bc[:], sel[:, b * 128 : (b + 1) * 128], A2[:],
                    start=True, stop=True,
                )
                t = ahbcpool.tile([128, HID], BF16, name=f"ahbc{b}", tag="ahbc")
                nc.scalar.copy(t[:], bc[:])
                ahbc[b] = t

            p_r = [
                p_d[b].rearrange("(g q j) h -> g q j h", q=128, j=J)
                for b in range(B_LOC)
            ]
            att_r = [
                att_d[b].rearrange("(g q j) h -> g q j h", q=128, j=J)
                for b in range(B_LOC)
            ]

            wgtT = {}

            def emit_scores(b):
                sc_b = sctpool.tile([128, ST], F32, name=f"sc{b}", tag="sc")
                for g in range(NG):
                    pt = ppool.tile([128, J, HID], BF16, name=f"pt{b}_{g}", tag="pt")
                    nc.sync.dma_start(pt[:], p_r[b][g])
                    nc.vector.tensor_add(
                        pt[:], pt[:],
                        ahbc[b][:, None, :].broadcast_to((128, J, HID)),
                    )
                    th = thpool.tile([128, J, HID], BF16, name=f"th{b}_{g}", tag="th")
                    nc.scalar.activation(
                        th[:], pt[:], mybir.ActivationFunctionType.Tanh
                    )
                    nc.vector.tensor_mul(
                        th[:], th[:],
                        wab[:, None, :].broadcast_to((128, J, HID)),
                    )
                    nc.vector.reduce_sum(
                        sc_b[:, ts(g, J)], th[:], axis=mybir.AxisListType.X
                    )
                wgt = wgtpool.tile([128, ST], BF16, name=f"wgt{b}", tag="wgt")
                nc.scalar.activation(
                    wgt[:], sc_b[:], mybir.ActivationFunctionType.Exp,
                    accum_out=zall[:, b : b + 1],
                )
                wgtT[b] = wgt

            def emit_weighted(b):
                accs = [
                    ps_acc.tile([1, 512], F32, name=f"acc{b}_{d}", tag="acc")
                    for d in range(DT)
                ]
                for g in range(NG):
                    for half in range(2):
                        at = attpool.tile(
                            [128, 2, D], BF16, name=f"at{b}_{g}_{half}", tag="at"
                        )
                        nc.sync.dma_start(
                            at[:], att_r[b][g][:, 2 * half : 2 * half + 2, :]
                        )
                        for u in range(2):
                            t = g * J + half * 2 + u
                            for d in range(DT):
                                nc.tensor.matmul(
                                    accs[d][:],
                                    wgtT[b][:, t : t + 1],
                                    at[:, u, ts(d, 512)],
                                    start=(t == 0),
                                    stop=(t == ST - 1),
                                )
                rowbuf = rowpool.tile([1, D], F32, name=f"row{b}", tag="row")
                for d in range(DT):
                    nc.scalar.copy(rowbuf[0:1, ts(d, 512)], accs[d][:])
                nc.scalar.dma_start(out_d[b : b + 1, :], rowbuf[:])

            # staggered emission: broadcasts and scores stay ~2 batches
            # ahead of the weighted-sum stream
            emit_bcast(0)
            emit_scores(0)
            emit_bcast(1)
            emit_scores(1)
            for b in range(B_LOC):
                if b + 2 < B_LOC:
                    emit_bcast(b + 2)
                    emit_scores(b + 2)
                emit_weighted(b)

            nc.scalar.dma_start(z_d[:], zall[:])

    nc.compile()
    return nc


def _in_maps(h, att_feats, p_att_feats, W_h2att, b_h2att, w_alpha):
    bf = ml_dtypes.bfloat16
    att_bf = np.ascontiguousarray(att_feats).astype(bf)
    p_bf = np.ascontiguousarray(p_att_feats).astype(bf)
    WT = np.ascontiguousarray(W_h2att.T).astype(bf)
    wab = np.ascontiguousarray(
        np.broadcast_to(w_alpha.astype(np.float32), (128, HID))
    ).astype(bf)
    bias8 = np.ascontiguousarray(
        np.broadcast_to(b_h2att.astype(np.float32), (B_LOC, HID))
    )
    sel = np.kron(
        np.eye(B_LOC, dtype=np.float32), np.ones((1, 128), dtype=np.float32)
    ).astype(bf)
    maps = []
    for c in range(8):
        sl = slice(c * B_LOC, (c + 1) * B_LOC)
        maps.append(
            {
                "p": np.ascontiguousarray(p_bf[sl]),
                "att": np.ascontiguousarray(att_bf[sl]),
                "hT": np.ascontiguousarray(h[sl].T.astype(bf)),
                "WT": WT,
                "wab": wab,
                "bias8": bias8,
                "sel": sel,
            }
        )
    return maps


def kernel(h, att_feats, p_att_feats, W_h2att, b_h2att, w_alpha, b_alpha):
    global _NC_CACHE
    h = np.asarray(h)
    att_feats = np.asarray(att_feats)
    p_att_feats = np.asarray(p_att_feats)
    W_h2att = np.asarray(W_h2att)
    b_h2att = np.asarray(b_h2att)
    w_alpha = np.asarray(w_alpha)
    if _NC_CACHE is None:
        _NC_CACHE = build_kernel()
    nc = _NC_CACHE
    maps = _in_maps(h, att_feats, p_att_feats, W_h2att, b_h2att, w_alpha)
    res = run_bass_kernel_spmd(nc, maps, core_ids=list(range(8)))
    outs = []
    for c in range(8):
        row = res.results[c]["out"]                     # [8, 2048] unnormalized
        z = res.results[c]["zall"].sum(axis=0)          # [8]
        outs.append(row / z[:, None])
    return np.concatenate(outs, axis=0).astype(np.float32)


# revision 10
# speedup vs baseline: 2.0066x; 1.0108x over previous
"""Trainium2 Bass kernel for the attention module:

    att_h  = h @ W_h2att.T + b_h2att             # [B, 512]
    dot    = tanh(p_att_feats + att_h[:, None])  # [B, 1024, 512]
    scores = dot @ w_alpha + b_alpha             # [B, 1024]
    weight = softmax(scores, axis=1)
    out    = einsum('bs,bsd->bd', weight, att_feats)  # [B, 2048]

Sharding: data-parallel over batch B=64 across 8 NeuronCores (8 per core).
Params are tiny and replicated. b_alpha is a softmax shift -> dropped.

v2c design (DMA-bound problem; ~44MB/core HBM read):
  - All large tensors (p, att, W) are cast to bf16 on the host; rel err
    budget is 2e-2 and the all-bf16 pipeline measures ~3e-3.
  - All input DMAs on the SP HWDGE ring (sync engine is otherwise idle,
    so DMA issue never blocks compute); tiny output DMAs on the ACT ring.
  - scores: DVE in-place add (bf16) -> ACT tanh (bf16) -> DVE in-place
    mul by w_alpha -> DVE reduce_sum into score columns.
  - softmax without max-subtraction (|scores| <= ||w_alpha||_1 ~ 18, exp
    can't overflow fp32): one ACT Exp per batch emits the bf16 weight
    tile (already in matmul lhsT layout) and f32 per-partition partial
    sums into a column of zall; Z-reduction and 1/Z normalization happen
    on the host (64 divides).
  - att_h row-broadcast via selector-matrix TensorE matmul (no DRAM
    bounce, no gpsimd).
  - weighted sum: per-b M=1 PSUM-accumulating bf16 matmuls; unnormalized
    rows + zall DMA'd out.

s-index mapping (shared by p, scores, weights, att):
    s = g*512 + q*4 + j   (g: group 0..1, q: partition 0..127, j: 0..3)
"""

import numpy as np
import ml_dtypes

import concourse.bass as bass
import concourse.tile as tile
from concourse import bacc, mybir
from concourse.bass import ts
from concourse.bass_utils import run_bass_kernel_spmd

F32 = mybir.dt.float32
BF16 = mybir.dt.bfloat16

B_LOC = 8       # batches per core
S = 1024        # attended positions
J = 4           # s per (group, partition)
NG = 2          # s groups
ST = NG * J     # 8 score columns
HID = 512
D = 2048
DT = D // 512   # 4 output column slices
K = 2048        # rnn_size (contraction for att_h)
KJ = 2
KG = K // (128 * KJ)  # 8 k-groups

_NC_CACHE = None


def build_kernel(att_bufs=12, p_bufs=6):
    nc = bacc.Bacc("TRN2", target_bir_lowering=False, debug=False, num_devices=8)

    p_d = nc.dram_tensor("p", [B_LOC, S, HID], BF16, kind="ExternalInput")
    att_d = nc.dram_tensor("att", [B_LOC, S, D], BF16, kind="ExternalInput")
    hT_d = nc.dram_tensor("hT", [K, B_LOC], BF16, kind="ExternalInput")
    WT_d = nc.dram_tensor("WT", [K, HID], BF16, kind="ExternalInput")
    wab_d = nc.dram_tensor("wab", [128, HID], BF16, kind="ExternalInput")
    bias8_d = nc.dram_tensor("bias8", [B_LOC, HID], F32, kind="ExternalInput")
    sel_d = nc.dram_tensor("sel", [B_LOC, B_LOC * 128], BF16, kind="ExternalInput")
    out_d = nc.dram_tensor("out", [B_LOC, D], F32, kind="ExternalOutput")
    z_d = nc.dram_tensor("zall", [128, B_LOC], F32, kind="ExternalOutput")

    with tile.TileContext(nc) as tc:
        with (
            tc.tile_pool(name="consts", bufs=1) as consts,
            tc.tile_pool(name="singles", bufs=1) as singles,
            tc.tile_pool(name="ahbc", bufs=B_LOC) as ahbcpool,
            tc.tile_pool(name="pp", bufs=p_bufs) as ppool,
            tc.tile_pool(name="th", bufs=3) as thpool,
            tc.tile_pool(name="sct", bufs=3) as sctpool,
            tc.tile_pool(name="wgtp", bufs=3) as wgtpool,
            tc.tile_pool(name="row", bufs=2) as rowpool,
            tc.tile_pool(name="attp", bufs=att_bufs) as attpool,
            tc.tile_pool(name="ps_setup", bufs=1, space=bass.MemorySpace.PSUM) as ps_setup,
            tc.tile_pool(name="ps_bc", bufs=1, space=bass.MemorySpace.PSUM) as ps_bc,
            tc.tile_pool(name="ps_acc", bufs=6, space=bass.MemorySpace.PSUM) as ps_acc,
        ):
            # ---- setup DMAs (SP ring, ahead of the p/att stream) ----
            wt_all = consts.tile([128, KG, KJ, HID], BF16)
            nc.sync.dma_start(
                wt_all[:], WT_d.rearrange("(kg q j) h -> q kg j h", q=128, j=KJ)
            )
            ht_all = consts.tile([128, KG, KJ, B_LOC], BF16)
            nc.sync.dma_start(
                ht_all[:], hT_d.rearrange("(kg q j) h -> q kg j h", q=128, j=KJ)
            )
            sel = consts.tile([B_LOC, B_LOC * 128], BF16)
            nc.sync.dma_start(sel[:], sel_d[:])
            wab = consts.tile([128, HID], BF16)
            nc.sync.dma_start(wab[:], wab_d[:])
            bias8 = consts.tile([B_LOC, HID], F32)
            nc.sync.dma_start(bias8[:], bias8_d[:])

            # ---- att_h = h @ W.T + b  ([8, 512]) ----
            atth_ps = ps_setup.tile([B_LOC, HID], F32)
            for kg in range(KG):
                for j in range(KJ):
                    nc.tensor.matmul(
                        atth_ps[:], ht_all[:, kg, j, :], wt_all[:, kg, j, :],
                        start=(kg == 0 and j == 0),
                        stop=(kg == KG - 1 and j == KJ - 1),
                    )
            A2 = singles.tile([B_LOC, HID], BF16)
            nc.vector.tensor_add(A2[:], atth_ps[:], bias8[:])

            # per-partition exp partial sums, one column per batch
            zall = singles.tile([128, B_LOC], F32)

            ahbc = [None] * B_LOC

            def emit_bcast(b):
                # broadcast att_h row b across 128 partitions: sel_b.T @ A2
                bc = ps_bc.tile([128, HID], F32, name=f"bc{b}", tag="bc")
                nc.tensor.matmul(
                    bc[:], sel[:, b * 128 : (b + 1) * 128], A2[:],
                    start=True, stop=True,
                )
                t = ahbcpool.tile([128, HID], BF16, name=f"ahbc{b}", tag="ahbc")
                nc.scalar.copy(t[:], bc[:])
                ahbc[b] = t

            p_r = [
                p_d[b].rearrange("(g q j) h -> g q j h", q=128, j=J)
                for b in range(B_LOC)
            ]
            att_r = [
                att_d[b].rearrange("(g q j) h -> g q j h", q=128, j=J)
                for b in range(B_LOC)
            ]

            wgtT = {}

            def emit_scores(b):
                sc_b = sctpool.tile([128, ST], F32, name=f"sc{b}", tag="sc")
                for g in range(NG):
                    pt = ppool.tile([128, J, HID], BF16, name=f"pt{b}_{g}", tag="pt")
                    nc.sync.dma_start(pt[:], p_r[b][g])
                    nc.vector.tensor_add(
                        pt[:], pt[:],
                        ahbc[b][:, None, :].broadcast_to((128, J, HID)),
                    )
                    th = thpool.tile([128, J, HID], BF16, name=f"th{b}_{g}", tag="th")
                    nc.scalar.activation(
                        th[:], pt[:], mybir.ActivationFunctionType.Tanh
                    )
                    nc.vector.tensor_mul(
                        th[:], th[:],
                        wab[:, None, :].broadcast_to((128, J, HID)),
                    )
                    nc.vector.reduce_sum(
                        sc_b[:, ts(g, J)], th[:], axis=mybir.AxisListType.X
                    )
                wgt = wgtpool.tile([128, ST], BF16, name=f"wgt{b}", tag="wgt")
                nc.scalar.activation(
                    wgt[:], sc_b[:], mybir.ActivationFunctionType.Exp,
                    accum_out=zall[:, b : b + 1],
                )
                wgtT[b] = wgt

            def emit_weighted(b):
                accs = [
                    ps_acc.tile([1, 512], F32, name=f"acc{b}_{d}", tag="acc")
                    for d in range(DT)
                ]
                for g in range(NG):
                    for half in range(2):
                        at = attpool.tile(
                            [128, 2, D], BF16, name=f"at{b}_{g}_{half}", tag="at"
                        )
                        nc.sync.dma_start(
                            at[:], att_r[b][g][:, 2 * half : 2 * half + 2, :]
                        )
                        for u in range(2):
                            t = g * J + half * 2 + u
                            for d in range(DT):
                                nc.tensor.matmul(
                                    accs[d][:],
                                    wgtT[b][:, t : t + 1],
                                    at[:, u, ts(d, 512)],
                                    start=(t == 0),
                                    stop=(t == ST - 1),
                                )
                rowbuf = rowpool.tile([1, D], F32, name=f"row{b}", tag="row")
                for d in range(DT):
                    nc.scalar.copy(rowbuf[0:1, ts(d, 512)], accs[d][:])
                nc.scalar.dma_start(out_d[b : b + 1, :], rowbuf[:])

            # staggered emission: broadcasts and scores stay ~2 batches
            # ahead of the weighted-sum stream
            emit_bcast(0)
            emit_scores(0)
            emit_bcast(1)
            emit_scores(1)
            for b in range(B_LOC):
                emit_weighted(b)
                if b + 2 < B_LOC:
                    emit_bcast(b + 2)
                    emit_scores(b + 2)

            nc.scalar.dma_start(z_d[:], zall[:])

    nc.compile()
    return nc


def _in_maps(h, att_feats, p_att_feats, W_h2att, b_h2att, w_alpha):
    bf = ml_dtypes.bfloat16
    att_bf = np.ascontiguousarray(att_feats).astype(bf)
    p_bf = np.ascontiguousarray(p_att_feats).astype(bf)
    WT = np.ascontiguousarray(W_h2att.T).astype(bf)
    wab = np.ascontiguousarray(
        np.broadcast_to(w_alpha.astype(np.float32), (128, HID))
    ).astype(bf)
    bias8 = np.ascontiguousarray(
        np.broadcast_to(b_h2att.astype(np.float32), (B_LOC, HID))
    )
    sel = np.kron(
        np.eye(B_LOC, dtype=np.float32), np.ones((1, 128), dtype=np.float32)
    ).astype(bf)
    maps = []
    for c in range(8):
        sl = slice(c * B_LOC, (c + 1) * B_LOC)
        maps.append(
            {
                "p": np.ascontiguousarray(p_bf[sl]),
                "att": np.ascontiguousarray(att_bf[sl]),
                "hT": np.ascontiguousarray(h[sl].T.astype(bf)),
                "WT": WT,
                "wab": wab,
                "bias8": bias8,
                "sel": sel,
            }
        )
    return maps


def kernel(h, att_feats, p_att_feats, W_h2att, b_h2att, w_alpha, b_alpha):
    global _NC_CACHE
    h = np.asarray(h)
    att_feats = np.asarray(att_feats)
    p_att_feats = np.asarray(p_att_feats)
    W_h2att = np.asarray(W_h2att)
    b_h2att = np.asarray(b_h2att)
    w_alpha = np.asarray(w_alpha)
    if _NC_CACHE is None:
        _NC_CACHE = build_kernel()
    nc = _NC_CACHE
    maps = _in_maps(h, att_feats, p_att_feats, W_h2att, b_h2att, w_alpha)
    res = run_bass_kernel_spmd(nc, maps, core_ids=list(range(8)))
    outs = []
    for c in range(8):
        row = res.results[c]["out"]                     # [8, 2048] unnormalized
        z = res.results[c]["zall"].sum(axis=0)          # [8]
        outs.append(row / z[:, None])
    return np.concatenate(outs, axis=0).astype(np.float32)


# revision 11
# speedup vs baseline: 2.0376x; 1.0154x over previous
"""Trainium2 Bass kernel for the attention module:

    att_h  = h @ W_h2att.T + b_h2att             # [B, 512]
    dot    = tanh(p_att_feats + att_h[:, None])  # [B, 1024, 512]
    scores = dot @ w_alpha + b_alpha             # [B, 1024]
    weight = softmax(scores, axis=1)
    out    = einsum('bs,bsd->bd', weight, att_feats)  # [B, 2048]

Sharding: data-parallel over batch B=64 across 8 NeuronCores (8 per core).
Params are tiny and replicated. b_alpha is a softmax shift -> dropped.

v2f design (DMA-bound; ~40MB/core HBM read, 16 SDMA engines saturated):
  - att/W in bf16, p in fp8-e4m3 (host-cast). Measured rel err ~1.3e-2
    against the fp32 reference (budget 2e-2). Set P_FP8=False to fall
    back to bf16 p (~3.3e-3).
  - All input DMAs on the SP HWDGE ring; ring order starts the big att
    stream immediately (att(0) before the 2MB W load) so the SDMA
    engines never idle during the prologue.
  - scores: DVE add -> ACT tanh (bf16) -> DVE in-place mul by
    w_alpha -> DVE reduce_sum into score columns.
  - softmax without max-subtraction (|scores| <= ||w_alpha||_1 ~ 18):
    one ACT Exp per batch emits the bf16 weight tile (matmul lhsT
    layout) and f32 per-partition partial sums into a column of zall;
    Z-reduction and 1/Z normalization happen on the host (64 divides).
  - att_h row-broadcast via selector-matrix TensorE matmul.
  - weighted sum: per-b M=1 PSUM-accumulating bf16 matmuls (1MB att
    tiles; 512KB for the last batch to shrink the drain tail); acc-copy
    emission precedes the next batch's tanh so PSUM banks free fast.

s-index mapping (shared by p, scores, weights, att):
    s = g*512 + q*4 + j   (g: group 0..1, q: partition 0..127, j: 0..3)
"""

import numpy as np
import ml_dtypes

import concourse.bass as bass
import concourse.tile as tile
from concourse import bacc, mybir
from concourse.bass import ts
from concourse.bass_utils import run_bass_kernel_spmd

F32 = mybir.dt.float32
BF16 = mybir.dt.bfloat16
F8 = mybir.dt.float8e4

P_FP8 = True
P_DT = F8 if P_FP8 else BF16

B_LOC = 8       # batches per core
S = 1024        # attended positions
J = 4           # s per (group, partition)
NG = 2          # s groups
ST = NG * J     # 8 score columns
HID = 512
D = 2048
DT = D // 512   # 4 output column slices
K = 2048        # rnn_size (contraction for att_h)
KJ = 2
KG = K // (128 * KJ)  # 8 k-groups

_NC_CACHE = None


def build_kernel(att_bufs=12, p_bufs=6):
    nc = bacc.Bacc("TRN2", target_bir_lowering=False, debug=False, num_devices=8)

    p_d = nc.dram_tensor("p", [B_LOC, S, HID], P_DT, kind="ExternalInput")
    att_d = nc.dram_tensor("att", [B_LOC, S, D], BF16, kind="ExternalInput")
    hT_d = nc.dram_tensor("hT", [K, B_LOC], BF16, kind="ExternalInput")
    WT_d = nc.dram_tensor("WT", [K, HID], BF16, kind="ExternalInput")
    wab_d = nc.dram_tensor("wab", [128, HID], BF16, kind="ExternalInput")
    bias8_d = nc.dram_tensor("bias8", [B_LOC, HID], F32, kind="ExternalInput")
    sel_d = nc.dram_tensor("sel", [B_LOC, B_LOC * 128], BF16, kind="ExternalInput")
    out_d = nc.dram_tensor("out", [B_LOC, D], F32, kind="ExternalOutput")
    z_d = nc.dram_tensor("zall", [128, B_LOC], F32, kind="ExternalOutput")

    with tile.TileContext(nc) as tc:
        with (
            tc.tile_pool(name="consts", bufs=1) as consts,
            tc.tile_pool(name="singles", bufs=1) as singles,
            tc.tile_pool(name="ahbc", bufs=B_LOC) as ahbcpool,
            tc.tile_pool(name="pp", bufs=p_bufs) as ppool,
            tc.tile_pool(name="pb", bufs=3) as pbpool,
            tc.tile_pool(name="th", bufs=3) as thpool,
            tc.tile_pool(name="sct", bufs=3) as sctpool,
            tc.tile_pool(name="wgtp", bufs=3) as wgtpool,
            tc.tile_pool(name="row", bufs=2) as rowpool,
            tc.tile_pool(name="attp", bufs=att_bufs) as attpool,
            tc.tile_pool(name="ps_setup", bufs=1, space=bass.MemorySpace.PSUM) as ps_setup,
            tc.tile_pool(name="ps_bc", bufs=1, space=bass.MemorySpace.PSUM) as ps_bc,
            tc.tile_pool(name="ps_acc", bufs=6, space=bass.MemorySpace.PSUM) as ps_acc,
        ):
            p_r = [
                p_d[b].rearrange("(g q j) h -> g q j h", q=128, j=J)
                for b in range(B_LOC)
            ]
            att_r = [
                att_d[b].rearrange("(g q j) h -> g q j h", q=128, j=J)
                for b in range(B_LOC)
            ]

            att_tiles = {}

            def emit_att_dma(b):
                # 1MB tiles (j-pairs); 512KB single-j tiles for the last
                # batch so the final drain is finer-grained
                tiles = []
                js = 1 if b == B_LOC - 1 else 2
                for g in range(NG):
                    for half in range(J // js):
                        at = attpool.tile(
                            [128, js, D], BF16, name=f"at{b}_{g}_{half}", tag="at"
                        )
                        nc.sync.dma_start(
                            at[:], att_r[b][g][:, js * half : js * (half + 1), :]
                        )
                        tiles.append((g, half, js, at))
                att_tiles[b] = tiles

            def emit_p_dma(b):
                tiles = []
                for g in range(NG):
                    pt = ppool.tile([128, J, HID], P_DT, name=f"pt{b}_{g}", tag="pt")
                    nc.sync.dma_start(pt[:], p_r[b][g])
                    tiles.append(pt)
                return tiles

            # ---- ring order: tiny consts, att(0), p(0), then W ----
            ht_all = consts.tile([128, KG, KJ, B_LOC], BF16)
            nc.sync.dma_start(
                ht_all[:], hT_d.rearrange("(kg q j) h -> q kg j h", q=128, j=KJ)
            )
            sel = consts.tile([B_LOC, B_LOC * 128], BF16)
            nc.sync.dma_start(sel[:], sel_d[:])
            wab = consts.tile([128, HID], BF16)
            nc.sync.dma_start(wab[:], wab_d[:])
            bias8 = consts.tile([B_LOC, HID], F32)
            nc.sync.dma_start(bias8[:], bias8_d[:])

            emit_att_dma(0)
            p_tiles = {0: emit_p_dma(0)}

            wt_all = consts.tile([128, KG, KJ, HID], BF16)
            nc.sync.dma_start(
                wt_all[:], WT_d.rearrange("(kg q j) h -> q kg j h", q=128, j=KJ)
            )

            # ---- att_h = h @ W.T + b  ([8, 512]) ----
            atth_ps = ps_setup.tile([B_LOC, HID], F32)
            for kg in range(KG):
                for j in range(KJ):
                    nc.tensor.matmul(
                        atth_ps[:], ht_all[:, kg, j, :], wt_all[:, kg, j, :],
                        start=(kg == 0 and j == 0),
                        stop=(kg == KG - 1 and j == KJ - 1),
                    )
            A2 = singles.tile([B_LOC, HID], BF16)
            nc.vector.tensor_add(A2[:], atth_ps[:], bias8[:])

            # per-partition exp partial sums, one column per batch
            zall = singles.tile([128, B_LOC], F32)

            ahbc = [None] * B_LOC

            def emit_bcast(b):
                # broadcast att_h row b across 128 partitions: sel_b.T @ A2
                bc = ps_bc.tile([128, HID], F32, name=f"bc{b}", tag="bc")
                nc.tensor.matmul(
                    bc[:], sel[:, b * 128 : (b + 1) * 128], A2[:],
                    start=True, stop=True,
                )
                t = ahbcpool.tile([128, HID], BF16, name=f"ahbc{b}", tag="ahbc")
                nc.scalar.copy(t[:], bc[:])
                ahbc[b] = t

            wgtT = {}

            def emit_scores(b):
                sc_b = sctpool.tile([128, ST], F32, name=f"sc{b}", tag="sc")
                for g in range(NG):
                    pt = p_tiles[b][g]
                    pb = pbpool.tile([128, J, HID], BF16, name=f"pb{b}_{g}", tag="pb")
                    nc.vector.tensor_add(
                        pb[:], pt[:],
                        ahbc[b][:, None, :].broadcast_to((128, J, HID)),
                    )
                    th = thpool.tile([128, J, HID], BF16, name=f"th{b}_{g}", tag="th")
                    nc.scalar.activation(
                        th[:], pb[:], mybir.ActivationFunctionType.Tanh
                    )
                    nc.vector.tensor_mul(
                        th[:], th[:],
                        wab[:, None, :].broadcast_to((128, J, HID)),
                    )
                    nc.vector.reduce_sum(
                        sc_b[:, ts(g, J)], th[:], axis=mybir.AxisListType.X
                    )
                wgt = wgtpool.tile([128, ST], BF16, name=f"wgt{b}", tag="wgt")
                nc.scalar.activation(
                    wgt[:], sc_b[:], mybir.ActivationFunctionType.Exp,
                    accum_out=zall[:, b : b + 1],
                )
                wgtT[b] = wgt

            def emit_weighted(b):
                accs = [
                    ps_acc.tile([1, 512], F32, name=f"acc{b}_{d}", tag="acc")
                    for d in range(DT)
                ]
                for g, half, js, at in att_tiles[b]:
                    for u in range(js):
                        t = g * J + half * js + u
                        for d in range(DT):
                            nc.tensor.matmul(
                                accs[d][:],
                                wgtT[b][:, t : t + 1],
                                at[:, u, ts(d, 512)],
                                start=(t == 0),
                                stop=(t == ST - 1),
                            )
                rowbuf = rowpool.tile([1, D], F32, name=f"row{b}", tag="row")
                for d in range(DT):
                    nc.scalar.copy(rowbuf[0:1, ts(d, 512)], accs[d][:])
                nc.scalar.dma_start(out_d[b : b + 1, :], rowbuf[:])

            # prologue: scores for b=0,1 ahead of the weighted stream
            emit_bcast(0)
            emit_scores(0)
            p_tiles[1] = emit_p_dma(1)
            emit_bcast(1)
            emit_scores(1)
            for b in range(B_LOC):
                if b + 1 < B_LOC:
                    emit_att_dma(b + 1)
                emit_weighted(b)
                if b + 2 < B_LOC:
                    p_tiles[b + 2] = emit_p_dma(b + 2)
                    emit_bcast(b + 2)
                    emit_scores(b + 2)

            nc.scalar.dma_start(z_d[:], zall[:])

    nc.compile()
    return nc


def _in_maps(h, att_feats, p_att_feats, W_h2att, b_h2att, w_alpha):
    bf = ml_dtypes.bfloat16
    p_np = ml_dtypes.float8_e4m3fn if P_FP8 else bf
    att_bf = np.ascontiguousarray(att_feats).astype(bf)
    p_q = np.ascontiguousarray(p_att_feats).astype(p_np)
    WT = np.ascontiguousarray(W_h2att.T).astype(bf)
    wab = np.ascontiguousarray(
        np.broadcast_to(w_alpha.astype(np.float32), (128, HID))
    ).astype(bf)
    bias8 = np.ascontiguousarray(
        np.broadcast_to(b_h2att.astype(np.float32), (B_LOC, HID))
    )
    sel = np.kron(
        np.eye(B_LOC, dtype=np.float32), np.ones((1, 128), dtype=np.float32)
    ).astype(bf)
    maps = []
    for c in range(8):
        sl = slice(c * B_LOC, (c + 1) * B_LOC)
        maps.append(
            {
                "p": np.ascontiguousarray(p_q[sl]),
                "att": np.ascontiguousarray(att_bf[sl]),
                "hT": np.ascontiguousarray(h[sl].T.astype(bf)),
                "WT": WT,
                "wab": wab,
                "bias8": bias8,
                "sel": sel,
            }
        )
    return maps


def kernel(h, att_feats, p_att_feats, W_h2att, b_h2att, w_alpha, b_alpha):
    global _NC_CACHE
    h = np.asarray(h)
    att_feats = np.asarray(att_feats)
    p_att_feats = np.asarray(p_att_feats)
    W_h2att = np.asarray(W_h2att)
    b_h2att = np.asarray(b_h2att)
    w_alpha = np.asarray(w_alpha)
    if _NC_CACHE is None:
        _NC_CACHE = build_kernel()
    nc = _NC_CACHE
    maps = _in_maps(h, att_feats, p_att_feats, W_h2att, b_h2att, w_alpha)
    res = run_bass_kernel_spmd(nc, maps, core_ids=list(range(8)))
    outs = []
    for c in range(8):
        row = res.results[c]["out"]                     # [8, 2048] unnormalized
        z = res.results[c]["zall"].sum(axis=0)          # [8]
        outs.append(row / z[:, None])
    return np.concatenate(outs, axis=0).astype(np.float32)
